# revision 14
# baseline (speedup 1.0000x reference)
"""Trainium2 Bass kernel for the Converter photometry problem.

Computes out = -2.5*log10(l_target @ (trans_filter * w).T) where w are
trapezoid quadrature weights derived from lam.  Data-parallel over 8
NeuronCores: l_target is sharded along batch B; the (small) weighted
filter matrix is replicated.

This problem is memory-bound (A = [8192, 8192] dominates traffic), so
everything is arranged to keep the per-core DMA stream at the ~358 GB/s
HBM line rate with the PE comfortably trailing it:

  - A is pre-transposed and packed ON HOST into the on-chip layout
    [p=128, chunk=64, b=1024] as fp8 e4m3 (8 MB/core, half of fp16).
    The GEMM contraction (L) then sits directly on SBUF partitions --
    no PE transposes, no PSUM staging, no DVE eviction copies at all.
  - WT = (trans_filter * w * 4096).T packed the same way as fp8
    ([p, chunk, f], 1 MB).  The x4096 keeps the smallest weights well
    inside the fp8 normal range (min ~0.12, max ~25); it is divided
    back out by the Ln activation's input scale.  fp8 rounding of both
    operands averages out over K=8192: rel err ~6e-4 vs fp32 reference.
  - A streams in eight 1 MB slabs on the sync HWDGE ring (one
    128-partition DMA already stripes across all 16 SDMA engines;
    >=1MB transfers run near line rate).
  - Matmuls run in fp8 DoubleRow perf mode (2 k-rows per PE cell,
    FD=512 >= the 256 crossover): lhsT = wt pair [128, 2, 128],
    rhs = A pair [128, 2, 512], accumulating flux.T [128f, 512b] in
    fp32 PSUM, one bank per 512-wide batch superblock.  PE work
    ~19 us < ~26 us of DMA, so the stream stays DMA-bound.
  - s-major order within each slab lets acc0's eviction (Ln + scale)
    overlap acc1's final matmuls; output is written as fp16 [F, 1024].
Host reassembles/upcasts the full [B, F] fp32 output.
"""

import math

import numpy as np

B, L, F = 8192, 8192, 128
N_CORES = 8
NB = B // N_CORES  # batch rows per core
P = 128
KC = L // P  # 64 contraction chunks of 128 l-rows
SBLK = 512  # PSUM free dim per accumulator bank
SLAB = 8  # chunks per A-stream DMA slab (1 MB)
UNIT_F_NU = 1.0673e-02
LOG10_SCALE = -2.5 / math.log(10.0)
WT_SCALE = 4096.0

_CACHE = {}


def _build_nc(repeat=1):
    import concourse.bacc as bacc
    import concourse.mybir as mybir
    from concourse import tile

    f32 = mybir.dt.float32
    f16 = mybir.dt.float16
    f8 = mybir.dt.float8e4

    NSLAB = KC // SLAB  # 8 slabs
    PAIRS = SLAB // 2  # 4 chunk pairs per slab (DoubleRow eats 2 chunks)

    nc = bacc.Bacc(None, target_bir_lowering=False, debug=False)
    # Both operands arrive host-packed in the on-chip [p, chunk, x] layout
    # so every DMA moves long per-partition contiguous runs at line rate.
    a = nc.dram_tensor("a", [P, KC * NB], f8, kind="ExternalInput")
    wt = nc.dram_tensor("wt", [P, KC * F], f8, kind="ExternalInput")
    o = nc.dram_tensor("o", [F, NB], f16, kind="ExternalOutput")

    with tile.TileContext(nc) as tc:
        with (
            tc.tile_pool(name="const", bufs=1) as const_pool,
            tc.tile_pool(name="acc", bufs=4, space="PSUM") as acc_pool,
            tc.tile_pool(name="out", bufs=4) as out_pool,
        ):
            wt_sb = const_pool.tile([P, KC, F], f8)
            a_sb = const_pool.tile([P, KC, NB], f8)
            warm = const_pool.tile([P, 1], f32)
            nc.gpsimd.memset(warm[:], 1.0)

            a_r = a.rearrange("p (c b) -> p c b", b=NB)
            wt_r = wt.rearrange("p (c f) -> p c f", f=F)

            # Loop-invariant work, hoisted: weights are stationary across
            # bodies (and the wt_sb WAR would otherwise chain iteration i+1's
            # first DMA to iteration i's LAST matmul, stalling the A stream).
            # Warming ACT's Ln table here keeps LoadActFuncSet (~1.3us) out
            # of the eviction tail.
            nc.scalar.dma_start(wt_sb[:], wt_r)
            nc.scalar.activation(
                warm[:], warm[:], mybir.ActivationFunctionType.Ln
            )

            def body():
                for si in range(NSLAB):
                    c0 = si * SLAB
                    nc.sync.dma_start(
                        a_sb[:, c0 : c0 + SLAB, :], a_r[:, c0 : c0 + SLAB, :]
                    )

                acc = [
                    acc_pool.tile([P, SBLK], f32, name=f"acc{s}")
                    for s in range(2)
                ]
                for si in range(NSLAB):
                    for s in range(2):
                        for j in range(PAIRS):
                            kp = si * PAIRS + j
                            nc.tensor.matmul(
                                acc[s][:],
                                wt_sb[:, 2 * kp : 2 * kp + 2, :],
                                a_sb[:, 2 * kp : 2 * kp + 2,
                                     s * SBLK : (s + 1) * SBLK],
                                start=(si == 0 and j == 0),
                                stop=(si == NSLAB - 1 and j == PAIRS - 1),
                                perf_mode=mybir.MatmulPerfMode.DoubleRow,
                            )
                # Eviction: all Ln activations issue back-to-back on ACT
                # (acc0's overlap acc1's last matmuls thanks to the s-major
                # slab order), DVE muls trail, and the out DMAs go LAST on
                # the scalar ring so no sequencer blocks on a semaphore
                # before its engine work is done -- and the sync ring stays
                # free for the next body's A slabs.  The Ln input scale
                # divides out the x4096 weight pre-scale.
                out_sb = [
                    out_pool.tile([P, SBLK], f16, name=f"out{s}")
                    for s in range(2)
                ]
                for s in range(2):
                    for h in range(2):
                        sl = slice(h * (SBLK // 2), (h + 1) * (SBLK // 2))
                        nc.scalar.activation(
                            out_sb[s][:, sl], acc[s][:, sl],
                            mybir.ActivationFunctionType.Ln,
                            scale=1.0 / WT_SCALE,
                        )
                for s in range(2):
                    for h in range(2):
                        sl = slice(h * (SBLK // 2), (h + 1) * (SBLK // 2))
                        nc.vector.tensor_scalar_mul(
                            out_sb[s][:, sl], out_sb[s][:, sl], LOG10_SCALE
                        )
                for s in range(2):
                    nc.scalar.dma_start(
                        o[:, s * SBLK : (s + 1) * SBLK], out_sb[s][:]
                    )

            if repeat == 1:
                body()
            elif repeat < 0:  # unrolled variant (cost-model introspection)
                for _ in range(-repeat):
                    body()
            else:
                # Unroll 4 bodies per hardware-loop iteration: the For_i
                # back-edge forces conservative cross-iteration semaphores
                # that stall the A stream; unrolling amortizes that 4x
                # (measured best at 4; 8 gains nothing further).
                unroll = 4 if repeat % 4 == 0 else 1
                with tc.For_i(0, repeat // unroll, 1):
                    for _ in range(unroll):
                        body()

    nc.compile()
    return nc


def get_nc():
    if "nc" not in _CACHE:
        _CACHE["nc"] = _build_nc()
    return _CACHE["nc"]


def _f8(x):
    import ml_dtypes

    return x.astype(ml_dtypes.float8_e4m3)


def make_weighted_filter_t(trans_filter, lam):
    """(trans_filter * trapz_weights * 4096).T as fp8 e4m3 in the on-chip
    [p, chunk, f] layout: element (p, c, f) = wt[c*128 + p, f]."""
    lam = np.asarray(lam, np.float32)
    tf = np.asarray(trans_filter, np.float32)
    dx = np.diff(lam)
    w = np.zeros(L, np.float32)
    w[:-1] += 0.5 * dx
    w[1:] += 0.5 * dx
    wt = (tf * (WT_SCALE * w)[None, :]).T  # [L, F] fp32
    wt = np.ascontiguousarray(wt.reshape(KC, P, F).transpose(1, 0, 2))
    return _f8(wt).reshape(P, KC * F)


def make_in_maps(l_target, trans_filter, lam):
    a8 = _f8(np.asarray(l_target, np.float32))  # [B, L] fp8
    # [core, p, chunk, b]: element (i, p, c, b) = A[i*NB + b, c*128 + p]
    ap = np.ascontiguousarray(
        a8.reshape(N_CORES, NB, KC, P).transpose(0, 3, 2, 1)
    ).reshape(N_CORES, P, KC * NB)
    wt8 = make_weighted_filter_t(trans_filter, lam)
    return [{"a": ap[i], "wt": wt8} for i in range(N_CORES)]


def kernel(l_target, trans_filter, lam, return_ph):
    rp = int(np.asarray(return_ph).reshape(()))
    if not rp:
        out = np.asarray(l_target, np.float32) * np.asarray(lam, np.float32)[None, :]
        return (out * np.float32(UNIT_F_NU)).astype(np.float32)

    from concourse.bass_utils import run_bass_kernel_spmd

    nc = get_nc()
    in_maps = make_in_maps(l_target, trans_filter, lam)
    res = run_bass_kernel_spmd(nc, in_maps, core_ids=list(range(N_CORES)))
    out = np.empty((B, F), np.float32)
    for i, r in enumerate(res.results):
        out[i * NB : (i + 1) * NB, :] = r["o"].T.astype(np.float32)
    return out


# revision 18
# speedup vs baseline: 1.0249x; 1.0249x over previous
"""Trainium2 Bass kernel for the Converter photometry problem.

Computes out = -2.5*log10(l_target @ (trans_filter * w).T) where w are
trapezoid quadrature weights derived from lam.  Data-parallel over 8
NeuronCores: l_target is sharded along batch B; the (small) weighted
filter matrix is replicated.

This problem is memory-bound (A = [8192, 8192] dominates traffic), so
everything is arranged to keep the per-core DMA stream at the ~358 GB/s
HBM line rate with the PE comfortably trailing it:

  - A is pre-transposed and packed ON HOST into the on-chip layout
    [p=128, chunk=64, b=1024] as fp8 e4m3 (8 MB/core, half of fp16).
    The GEMM contraction (L) then sits directly on SBUF partitions --
    no PE transposes, no PSUM staging, no DVE eviction copies at all.
  - WT = (trans_filter * w * 4096).T packed the same way as fp8
    ([p, chunk, f], 1 MB).  The x4096 keeps the smallest weights well
    inside the fp8 normal range (min ~0.12, max ~25); it is divided
    back out by the Ln activation's input scale.  fp8 rounding of both
    operands averages out over K=8192: rel err ~6e-4 vs fp32 reference.
  - A streams in eight 1 MB slabs on the sync HWDGE ring (one
    128-partition DMA already stripes across all 16 SDMA engines;
    >=1MB transfers run near line rate).
  - Matmuls run in fp8 DoubleRow perf mode (2 k-rows per PE cell,
    FD=512 >= the 256 crossover): lhsT = wt pair [128, 2, 128],
    rhs = A pair [128, 2, 512], accumulating flux.T [128f, 512b] in
    fp32 PSUM, one bank per 512-wide batch superblock.  PE work
    ~19 us < ~26 us of DMA, so the stream stays DMA-bound.
  - s-major order within each slab lets acc0's eviction (Ln + scale)
    overlap acc1's final matmuls; output is written as fp16 [F, 1024].
Host reassembles/upcasts the full [B, F] fp32 output.
"""

import math

import numpy as np

B, L, F = 8192, 8192, 128
N_CORES = 8
NB = B // N_CORES  # batch rows per core
P = 128
KC = L // P  # 64 contraction chunks of 128 l-rows
SBLK = 512  # PSUM free dim per accumulator bank
SLAB = 8  # chunks per A-stream DMA slab (1 MB)
UNIT_F_NU = 1.0673e-02
LOG10_SCALE = -2.5 / math.log(10.0)
WT_SCALE = 4096.0

_CACHE = {}


def _build_nc(repeat=1):
    import concourse.bacc as bacc
    import concourse.mybir as mybir
    from concourse import tile

    f32 = mybir.dt.float32
    f16 = mybir.dt.float16
    f8 = mybir.dt.float8e4

    NSLAB = KC // SLAB  # 8 slabs
    PAIRS = SLAB // 2  # 4 chunk pairs per slab (DoubleRow eats 2 chunks)

    nc = bacc.Bacc(None, target_bir_lowering=False, debug=False)
    # Both operands arrive host-packed in the on-chip layout so every DMA
    # moves long per-partition contiguous runs.  A is additionally packed
    # slab-major ([slab, p, bytes]) so each 1 MB slab DMA reads one fully
    # CONTIGUOUS dram region (partition stride 8 KB, not 64 KB) for better
    # HBM row locality.
    a = nc.dram_tensor("a", [NSLAB * P, SLAB * NB], f8, kind="ExternalInput")
    wt = nc.dram_tensor("wt", [P, KC * F], f8, kind="ExternalInput")
    o = nc.dram_tensor("o", [F, NB], f16, kind="ExternalOutput")

    with tile.TileContext(nc) as tc:
        with (
            tc.tile_pool(name="const", bufs=1) as const_pool,
            tc.tile_pool(name="acc", bufs=4, space="PSUM") as acc_pool,
            tc.tile_pool(name="out", bufs=4) as out_pool,
        ):
            wt_sb = const_pool.tile([P, KC, F], f8)
            a_sb = const_pool.tile([P, KC, NB], f8)
            warm = const_pool.tile([P, 1], f32)
            nc.gpsimd.memset(warm[:], 1.0)

            a_r = a.rearrange("(s p) (c b) -> s p c b", p=P, b=NB)
            wt_r = wt.rearrange("p (c f) -> p c f", f=F)

            # Loop-invariant work, hoisted: weights are stationary across
            # bodies (and the wt_sb WAR would otherwise chain iteration i+1's
            # first DMA to iteration i's LAST matmul, stalling the A stream).
            # Warming ACT's Ln table here keeps LoadActFuncSet (~1.3us) out
            # of the eviction tail.
            nc.scalar.dma_start(wt_sb[:], wt_r)
            nc.scalar.activation(
                warm[:], warm[:], mybir.ActivationFunctionType.Ln
            )

            def body():
                for si in range(NSLAB):
                    c0 = si * SLAB
                    nc.sync.dma_start(
                        a_sb[:, c0 : c0 + SLAB, :], a_r[si]
                    )

                acc = [
                    acc_pool.tile([P, SBLK], f32, name=f"acc{s}")
                    for s in range(2)
                ]
                for si in range(NSLAB):
                    for s in range(2):
                        for j in range(PAIRS):
                            kp = si * PAIRS + j
                            nc.tensor.matmul(
                                acc[s][:],
                                wt_sb[:, 2 * kp : 2 * kp + 2, :],
                                a_sb[:, 2 * kp : 2 * kp + 2,
                                     s * SBLK : (s + 1) * SBLK],
                                start=(si == 0 and j == 0),
                                stop=(si == NSLAB - 1 and j == PAIRS - 1),
                                perf_mode=mybir.MatmulPerfMode.DoubleRow,
                            )
                # Eviction: all Ln activations issue back-to-back on ACT
                # (acc0's overlap acc1's last matmuls thanks to the s-major
                # slab order), DVE muls trail, and the out DMAs go LAST on
                # the scalar ring so no sequencer blocks on a semaphore
                # before its engine work is done -- and the sync ring stays
                # free for the next body's A slabs.  The Ln input scale
                # divides out the x4096 weight pre-scale.
                out_sb = [
                    out_pool.tile([P, SBLK], f16, name=f"out{s}")
                    for s in range(2)
                ]
                for s in range(2):
                    for h in range(2):
                        sl = slice(h * (SBLK // 2), (h + 1) * (SBLK // 2))
                        nc.scalar.activation(
                            out_sb[s][:, sl], acc[s][:, sl],
                            mybir.ActivationFunctionType.Ln,
                            scale=1.0 / WT_SCALE,
                        )
                for s in range(2):
                    for h in range(2):
                        sl = slice(h * (SBLK // 2), (h + 1) * (SBLK // 2))
                        nc.vector.tensor_scalar_mul(
                            out_sb[s][:, sl], out_sb[s][:, sl], LOG10_SCALE
                        )
                for s in range(2):
                    nc.scalar.dma_start(
                        o[:, s * SBLK : (s + 1) * SBLK], out_sb[s][:]
                    )

            if repeat == 1:
                body()
            elif repeat < 0:  # unrolled variant (cost-model introspection)
                for _ in range(-repeat):
                    body()
            else:
                # Unroll 4 bodies per hardware-loop iteration: the For_i
                # back-edge forces conservative cross-iteration semaphores
                # that stall the A stream; unrolling amortizes that 4x
                # (measured best at 4; 8 gains nothing further).
                unroll = 4 if repeat % 4 == 0 else 1
                with tc.For_i(0, repeat // unroll, 1):
                    for _ in range(unroll):
                        body()

    nc.compile()
    return nc


def get_nc():
    if "nc" not in _CACHE:
        _CACHE["nc"] = _build_nc()
    return _CACHE["nc"]


def _f8(x):
    import ml_dtypes

    return x.astype(ml_dtypes.float8_e4m3)


def make_weighted_filter_t(trans_filter, lam):
    """(trans_filter * trapz_weights * 4096).T as fp8 e4m3 in the on-chip
    [p, chunk, f] layout: element (p, c, f) = wt[c*128 + p, f]."""
    lam = np.asarray(lam, np.float32)
    tf = np.asarray(trans_filter, np.float32)
    dx = np.diff(lam)
    w = np.zeros(L, np.float32)
    w[:-1] += 0.5 * dx
    w[1:] += 0.5 * dx
    wt = (tf * (WT_SCALE * w)[None, :]).T  # [L, F] fp32
    wt = np.ascontiguousarray(wt.reshape(KC, P, F).transpose(1, 0, 2))
    return _f8(wt).reshape(P, KC * F)


def make_in_maps(l_target, trans_filter, lam):
    a8 = _f8(np.asarray(l_target, np.float32))  # [B, L] fp8
    # Slab-major pack [core, slab, p, chunk_in_slab, b]: element
    # (i, s, p, c, b) = A[i*NB + b, (s*SLAB + c)*128 + p], so each 1 MB
    # slab is one contiguous dram region with partition stride SLAB*NB.
    NSLAB = KC // SLAB
    ap = np.ascontiguousarray(
        a8.reshape(N_CORES, NB, NSLAB, SLAB, P).transpose(0, 2, 4, 3, 1)
    ).reshape(N_CORES, NSLAB * P, SLAB * NB)
    wt8 = make_weighted_filter_t(trans_filter, lam)
    return [{"a": ap[i], "wt": wt8} for i in range(N_CORES)]


def kernel(l_target, trans_filter, lam, return_ph):
    rp = int(np.asarray(return_ph).reshape(()))
    if not rp:
        out = np.asarray(l_target, np.float32) * np.asarray(lam, np.float32)[None, :]
        return (out * np.float32(UNIT_F_NU)).astype(np.float32)

    from concourse.bass_utils import run_bass_kernel_spmd

    nc = get_nc()
    in_maps = make_in_maps(l_target, trans_filter, lam)
    res = run_bass_kernel_spmd(nc, in_maps, core_ids=list(range(N_CORES)))
    out = np.empty((B, F), np.float32)
    for i, r in enumerate(res.results):
        out[i * NB : (i + 1) * NB, :] = r["o"].T.astype(np.float32)
    return out
